# revision 5
# baseline (speedup 1.0000x reference)
import sys

sys.path.insert(0, "/opt/trn_rl_repo")

import numpy as np

import concourse.bass as bass
from concourse import bacc
import concourse.mybir as mybir
import concourse.tile as tile
from concourse.bass import ts
from concourse.bass_utils import run_bass_kernel_spmd

B, DIM, H, W = 2, 128, 128, 128
GC, NSET, KS = 2, 16, 3
G = DIM // GC
KK = KS * KS
INTERC = 16

NCORES = 8
HB = 4            # h-stripes per batch  (8 cores = 2 batches x 4 stripes)
RH = H // HB      # 32 output rows per core
SH = RH + 4       # 36 shard rows (halo 2 each side)
WP = W + 2        # 130 padded width
NPIX = SH * WP    # 4680
NOUT = RH * WP    # 4160 (output grid incl pad cols)
ET = 416          # einsum tile width
NT = NOUT // ET   # 10

F32 = mybir.dt.float32
BF16 = mybir.dt.bfloat16

_NC_CACHE = {}
_LAST_IN_MAPS = None


def _build_nc():
    nc = bacc.Bacc(None, target_bir_lowering=False, debug=False)
    p = {}

    def inp(name, shape):
        p[name] = nc.declare_dram_parameter(name, list(shape), F32, isOutput=False)

    inp("x", (DIM, NPIX))
    inp("mask", (1, NPIX))
    inp("w1pwT", (DIM, DIM))
    inp("b1pw", (1, DIM))
    inp("dwm", (DIM, 9 * DIM))
    inp("b1dw", (1, DIM))
    inp("w2g", (DIM, 9 * INTERC))
    inp("b2g", (1, INTERC))
    inp("w211", (DIM, INTERC))
    inp("w2pw", (INTERC // 2, INTERC))
    inp("battn", (1, INTERC))
    inp("selfb", (NSET, DIM))
    inp("selfwT", (NSET, 18 * DIM))
    inp("iden", (DIM, DIM))
    inp("s0", (DIM, DIM))
    inp("s1", (DIM, DIM))
    inp("ga1", (DIM, 1))
    out_p = nc.declare_dram_parameter("out", [DIM, RH * W], F32, isOutput=True)

    CP = mybir.ActivationFunctionType.Copy

    with tile.TileContext(nc) as tc:
        with tc.tile_pool(name="const", bufs=1) as cpool, \
             tc.tile_pool(name="big", bufs=1) as bpool, \
             tc.tile_pool(name="tprod", bufs=3) as tpool, \
             tc.tile_pool(name="psA", bufs=3, space="PSUM") as psA, \
             tc.tile_pool(name="psJ", bufs=3, space="PSUM") as psJ, \
             tc.tile_pool(name="psY", bufs=2, space="PSUM") as psY:

            def csb(name, shape):
                t = cpool.tile(list(shape), F32, tag=name)
                nc.sync.dma_start(out=t[:], in_=p[name][:])
                return t

            w1pwT = csb("w1pwT", (DIM, DIM))
            b1pw = csb("b1pw", (1, DIM))
            dwm = csb("dwm", (DIM, 9 * DIM))
            b1dw = csb("b1dw", (1, DIM))
            w2g = csb("w2g", (DIM, 9 * INTERC))
            b2g = csb("b2g", (1, INTERC))
            w211 = csb("w211", (DIM, INTERC))
            w2pw = csb("w2pw", (INTERC // 2, INTERC))
            battn = csb("battn", (1, INTERC))
            selfb = csb("selfb", (NSET, DIM))
            selfwT = csb("selfwT", (NSET, 18 * DIM))
            iden = csb("iden", (DIM, DIM))
            s0 = csb("s0", (DIM, DIM))
            s1 = csb("s1", (DIM, DIM))
            ga1 = csb("ga1", (DIM, 1))
            ones = cpool.tile([1, 512], F32, tag="ones")
            nc.vector.memset(ones[:], 1.0)

            x_sb = bpool.tile([DIM, NPIX], F32, tag="x")
            nc.sync.dma_start(out=x_sb[:], in_=p["x"][:])
            mask = bpool.tile([DIM, NPIX], F32, tag="mask")
            nc.sync.dma_start(out=mask[:], in_=p["mask"][:].to_broadcast([DIM, NPIX]))

            # ---- conv1_pw:  pwx = (W1 @ x + b1) * mask ----
            pwx = bpool.tile([DIM, NPIX], F32, tag="pwx")
            NCH = 10
            CW = NPIX // NCH  # 468
            for c in range(NCH):
                ps = psA.tile([DIM, 512], F32, tag="ps")
                nc.tensor.matmul(ps[:, :CW], w1pwT[:], x_sb[:, ts(c, CW)],
                                 start=True, stop=False)
                nc.tensor.matmul(ps[:, :CW], b1pw[:], ones[:, :CW],
                                 start=False, stop=True)
                nc.scalar.activation(pwx[:, ts(c, CW)], ps[:, :CW], CP)
            nc.vector.tensor_mul(pwx[:], pwx[:], mask[:])

            # ---- conv1_dw: 9 block-diag matmuls, out rows 1..34 of grid ----
            enh = bpool.tile([DIM, NPIX], F32, tag="enh")
            nc.vector.memset(enh[:], 0.0)
            dchunks = [(131 + 496 * k, 496) for k in range(8)] + [(131 + 3968, 450)]
            for (st, sz) in dchunks:
                ps = psA.tile([DIM, 512], F32, tag="ps")
                for kp in range(9):
                    dh, dw = kp // 3 - 1, kp % 3 - 1
                    off = st + dh * WP + dw
                    nc.tensor.matmul(ps[:, :sz], dwm[:, ts(kp, DIM)],
                                     pwx[:, off:off + sz],
                                     start=(kp == 0), stop=False)
                nc.tensor.matmul(ps[:, :sz], b1dw[:], ones[:, :sz],
                                 start=False, stop=True)
                nc.scalar.activation(enh[:, st:st + sz], ps[:, :sz], CP)
            nc.vector.tensor_mul(enh[:], enh[:], mask[:])

            # ---- enhE / enhO: even/odd channel duplication (bf16) ----
            enhE = bpool.tile([DIM, NPIX], BF16, tag="enhE")
            enhO = bpool.tile([DIM, NPIX], BF16, tag="enhO")
            for c in range(NCH):
                psE = psA.tile([DIM, 512], F32, tag="ps")
                nc.tensor.matmul(psE[:, :CW], s0[:], enh[:, ts(c, CW)],
                                 start=True, stop=True)
                nc.scalar.activation(enhE[:, ts(c, CW)], psE[:, :CW], CP)
                psO = psA.tile([DIM, 512], F32, tag="ps")
                nc.tensor.matmul(psO[:, :CW], s1[:], enh[:, ts(c, CW)],
                                 start=True, stop=True)
                nc.scalar.activation(enhO[:, ts(c, CW)], psO[:, :CW], CP)

            # ---- conv2_g (grouped 3x3, 16 out ch) on out grid ----
            h_sb = bpool.tile([INTERC, NOUT], F32, tag="h")
            ACH = 10
            AW = NOUT // ACH  # 416
            for c in range(ACH):
                ps = psA.tile([INTERC, 512], F32, tag="ps")
                base = 2 * WP + c * AW
                for kp in range(9):
                    dh, dw = kp // 3 - 1, kp % 3 - 1
                    off = base + dh * WP + dw
                    nc.tensor.matmul(ps[:, :AW], w2g[:, ts(kp, INTERC)],
                                     x_sb[:, off:off + AW],
                                     start=(kp == 0), stop=False)
                nc.tensor.matmul(ps[:, :AW], b2g[:], ones[:, :AW],
                                 start=False, stop=True)
                nc.scalar.activation(h_sb[:, ts(c, AW)], ps[:, :AW], CP)

            # ---- SimpleGate ----
            h2c = bpool.tile([INTERC // 2, NOUT], F32, tag="h2c")
            nc.sync.dma_start(out=h2c[:], in_=h_sb[8:16, :])
            g_sb = bpool.tile([INTERC // 2, NOUT], F32, tag="g")
            nc.vector.tensor_mul(g_sb[:], h_sb[0:8, :], h2c[:])

            # ---- attn:  att2 = gamma*conv2_pw(g) + conv211(x) + bias ----
            att2 = bpool.tile([NSET, NOUT], F32, tag="att2")
            for c in range(ACH):
                ps = psA.tile([NSET, 512], F32, tag="ps")
                base = 2 * WP + c * AW
                nc.tensor.matmul(ps[:, :AW], w2pw[:], g_sb[:, ts(c, AW)],
                                 start=True, stop=False)
                nc.tensor.matmul(ps[:, :AW], w211[:], x_sb[:, base:base + AW],
                                 start=False, stop=False)
                nc.tensor.matmul(ps[:, :AW], battn[:], ones[:, :AW],
                                 start=False, stop=True)
                nc.scalar.activation(att2[:, ts(c, AW)], ps[:, :AW], CP)

            # ---- KBA dynamic conv ----
            final = bpool.tile([DIM, NOUT], F32, tag="final")
            for t in range(NT):
                q0 = t * ET
                y_ps = psY.tile([DIM, ET], F32, tag="y")
                nc.tensor.matmul(y_ps[:], selfb[:], att2[:, q0:q0 + ET],
                                 start=True, stop=False)
                for j in range(18):
                    gcin, kp = j // 9, j % 9
                    dh, dw = kp // 3 - 1, kp % 3 - 1
                    src = enhE if gcin == 0 else enhO
                    off = q0 + (2 + dh) * WP + dw
                    psj = psJ.tile([DIM, ET], F32, tag="j")
                    nc.tensor.matmul(psj[:], selfwT[:, ts(j, DIM)],
                                     att2[:, q0:q0 + ET], start=True, stop=True)
                    tj = tpool.tile([DIM, ET], F32, tag="t")
                    nc.vector.tensor_mul(tj[:], psj[:], src[:, off:off + ET])
                    nc.tensor.matmul(y_ps[:], iden[:], tj[:],
                                     start=False, stop=(j == 17))
                nc.scalar.activation(final[:, q0:q0 + ET], y_ps[:], CP,
                                     scale=ga1[:])

            # ---- residuals ----
            nc.vector.tensor_add(final[:], final[:], enh[:, 2 * WP:2 * WP + NOUT])
            nc.vector.tensor_add(final[:], final[:], x_sb[:, 2 * WP:2 * WP + NOUT])

            fin3 = final[:].rearrange("p (r w) -> p r w", w=WP)
            nc.sync.dma_start(out=out_p[:], in_=fin3[:, :, 1:1 + W])

    if not nc.is_finalized():
        nc.finalize()
    return nc


def _get_nc():
    if "nc" not in _NC_CACHE:
        _NC_CACHE["nc"] = _build_nc()
    return _NC_CACHE["nc"]


def _prep_consts(ins):
    f = np.float32
    c = {}
    c["w1pwT"] = np.ascontiguousarray(ins["w_conv1_pw"][:, :, 0, 0].T).astype(f)
    c["b1pw"] = ins["b_conv1_pw"].reshape(1, DIM).astype(f)

    dwm = np.zeros((DIM, 9, DIM), f)
    for kp in range(9):
        di, dj = kp // 3, kp % 3
        np.fill_diagonal(dwm[:, kp, :], ins["w_conv1_dw"][:, 0, di, dj])
    c["dwm"] = dwm.reshape(DIM, 9 * DIM)
    c["b1dw"] = ins["b_conv1_dw"].reshape(1, DIM).astype(f)

    w2g = np.zeros((DIM, 9, INTERC), f)
    for co in range(INTERC):
        for ci in range(DIM // INTERC):
            for kp in range(9):
                di, dj = kp // 3, kp % 3
                w2g[8 * co + ci, kp, co] = ins["w_conv2_g"][co, ci, di, dj]
    c["w2g"] = w2g.reshape(DIM, 9 * INTERC)
    c["b2g"] = ins["b_conv2_g"].reshape(1, INTERC).astype(f)

    gam = ins["attgamma"][0, :, 0, 0].astype(f)  # [16]
    c["w211"] = np.ascontiguousarray(ins["w_conv211"][:, :, 0, 0].T).astype(f)
    c["w2pw"] = np.ascontiguousarray(
        (ins["w_conv2_pw"][:, :, 0, 0] * gam[:, None]).T).astype(f)
    c["battn"] = (gam * ins["b_conv2_pw"] + ins["b_conv211"]).reshape(1, INTERC).astype(f)

    c["selfb"] = np.ascontiguousarray(ins["selfb"][0]).astype(f)  # [16,128]
    sw = ins["selfw"][0].reshape(NSET, G, GC, GC * KK).astype(f)
    # chunk_j[n, 2g+i] = selfw[n, g, i, j]
    c["selfwT"] = np.ascontiguousarray(
        sw.transpose(0, 3, 1, 2).reshape(NSET, 18 * DIM))
    c["iden"] = np.eye(DIM, dtype=f)
    s0 = np.zeros((DIM, DIM), f)
    s0[(np.arange(DIM) // 2) * 2, np.arange(DIM)] = 1.0
    s1 = np.zeros((DIM, DIM), f)
    s1[(np.arange(DIM) // 2) * 2 + 1, np.arange(DIM)] = 1.0
    c["s0"], c["s1"] = s0, s1
    c["ga1"] = ins["ga1"][0, :, 0, 0].reshape(DIM, 1).astype(f)
    return c


def _make_in_maps(inputs):
    ins = {k: np.asarray(v, np.float32) for k, v in inputs.items()}
    consts = _prep_consts(ins)
    xp = np.pad(ins["x"], ((0, 0), (0, 0), (2, 2), (1, 1)))
    in_maps = []
    for core in range(NCORES):
        b, hb = core // HB, core % HB
        shard = np.ascontiguousarray(
            xp[b, :, RH * hb:RH * hb + SH, :]).reshape(DIM, NPIX)
        m = np.zeros((SH, WP), np.float32)
        for r in range(SH):
            gr = RH * hb + r - 2
            if 0 <= gr < H:
                m[r, 1:1 + W] = 1.0
        im = dict(consts)
        im["x"] = shard
        im["mask"] = m.reshape(1, NPIX)
        in_maps.append(im)
    return in_maps


def _assemble(results):
    outf = np.empty((B, DIM, H, W), np.float32)
    for core in range(NCORES):
        b, hb = core // HB, core % HB
        outf[b, :, RH * hb:RH * hb + RH, :] = \
            np.asarray(results[core]["out"]).reshape(DIM, RH, W)
    return outf


def kernel(**inputs):
    global _LAST_IN_MAPS
    in_maps = _make_in_maps(inputs)
    _LAST_IN_MAPS = in_maps
    nc = _get_nc()
    res = run_bass_kernel_spmd(nc, in_maps, core_ids=list(range(NCORES)))
    return _assemble(res.results)


def profile_exec_ns(inputs=None):
    """Run with NTFF tracing; return (exec_time_ns, results)."""
    global _LAST_IN_MAPS
    if inputs is not None:
        _LAST_IN_MAPS = _make_in_maps(inputs)
    assert _LAST_IN_MAPS is not None
    nc = _get_nc()
    try:
        res = run_bass_kernel_spmd(nc, _LAST_IN_MAPS, core_ids=list(range(NCORES)),
                                   trace=True)
        return res.exec_time_ns, res
    except Exception as e:
        print("trace unavailable:", repr(e)[:120])
        return None, None
